# revision 11
# baseline (speedup 1.0000x reference)
"""Binarized conv2d kernel for Trainium2, SPMD over 8 NeuronCores.

Math (forward-value equivalent of the reference):
    real_w  = sum_k RV[k] * weights[k]          # [256,256,3,3], exact fp32 on DVE
    scale   = mean(|real_w|, axis=(1,2,3))      # per out-channel
    out     = conv2d(sign(x), sign(real_w), pad=1) * (scale * alpha)

sign(x) and sign(real_w) are {-1,0,+1} which are exact in fp8e4, so the conv
is computed with fp8 DoubleRow matmuls (exact integer accumulation in fp32
PSUM) and the per-channel scale*alpha is applied on PSUM evacuation.

Sharding: data-parallel over batch, 4 images per core; weights/RV/alpha
replicated. No collectives (an 8-core AllGather measures ~80us on this
fabric — host-proxied — so TP weight prep is a net loss).

Schedule: the front of the kernel is DMA-bound (~22MB of weights+x reads at
the ~360-420GB/s per-core ceiling), so the conv passes are ordered h0 for
ALL four images first, then h1: the h1 weight half (4.7MB) is interleaved
into the x stream and only needed at ~60% of the kernel.  x images load in
two row bands so each image's first conv tile starts as soon as ~45% of its
pixels have landed.  Outputs are written bf16 (halves write traffic; well
inside the 2e-2 tolerance) and upcast to f32 on the host.
"""

import numpy as np
from contextlib import ExitStack

import concourse.bass as bass
import concourse.bacc as bacc
import concourse.tile as tile
from concourse import mybir
from concourse.bass_utils import run_bass_kernel_spmd
from concourse.masks import make_identity

# Problem shapes (hardcoded per contract)
B, C, H, W = 32, 256, 56, 56
K, KS = 4, 3
NCORES = 8
BL = B // NCORES            # images per core

PW = W + 2                  # padded width 58
PLANE = PW * PW             # 3364
PL = 3376                   # plane stride (>= 1+PLANE+1, multiple of 16)
GO = 1                      # guard offset: plane data starts at elem 1
RPC = 8                     # rows per chunk
CHUNK = RPC * PW            # 464 elems per matmul (one PSUM bank)
NCHUNK = H // RPC           # 7 chunks: first psum tile gets 3, second 4
CIH = C // 128              # 2 ci halves
COH = C // 128              # 2 co halves
TAPS = KS * KS              # 9

# rows of x covered by the first conv tile (chunks 0-2 + halo)
XA_ROWS = 3 * RPC + 1       # 25

F32 = mybir.dt.float32
FP8 = mybir.dt.float8e4
BF16 = mybir.dt.bfloat16

_cache = {}


def _build():
    act_dt = FP8
    nc = bacc.Bacc("TRN2", target_bir_lowering=False, debug=False,
                   num_devices=NCORES)
    x_d = nc.dram_tensor("x", [BL, C, H, W], F32, kind="ExternalInput")
    w_d = nc.dram_tensor("weights", [K, C, C, KS, KS], F32, kind="ExternalInput")
    rv_d = nc.dram_tensor("RV", [K + 1], F32, kind="ExternalInput")
    al_d = nc.dram_tensor("alpha", [C, 1, 1], F32, kind="ExternalInput")
    o_d = nc.dram_tensor("out", [BL, C, H, W], BF16, kind="ExternalOutput")

    with tile.TileContext(nc) as tc, ExitStack() as ctx:
        consts = ctx.enter_context(tc.tile_pool(name="consts", bufs=1))
        wstage = ctx.enter_context(tc.tile_pool(name="wstage", bufs=8))
        wwork = ctx.enter_context(tc.tile_pool(name="wwork", bufs=1))
        xin = ctx.enter_context(tc.tile_pool(name="xin", bufs=2))
        xpads = ctx.enter_context(tc.tile_pool(name="xpads", bufs=1))
        # 6 bufs: the 4 h0-pass outputs are held in SBUF until their
        # deferred DMAs fire, plus 2 rotating for the h1 pass
        outp = ctx.enter_context(tc.tile_pool(name="outp", bufs=6))

        # --- tiny constant loads on the ACT HWDGE ring ---------------------
        rv = consts.tile([128, K], F32, tag="rv")
        rv_src = bass.AP(tensor=rv_d.ap().tensor, offset=0,
                         ap=[[0, 128], [1, K]])
        nc.scalar.dma_start(out=rv, in_=rv_src)
        alpha_sb = []
        for h in range(COH):
            t = consts.tile([128, 1], F32, tag=f"alpha{h}")
            nc.scalar.dma_start(out=t,
                                in_=al_d.ap()[h * 128:(h + 1) * 128, 0, :])
            alpha_sb.append(t)

        # --- padded x planes (one per image): zero the pad borders on DVE --
        xpad = []
        for i in range(BL):
            t = xpads.tile([128, CIH, PL], act_dt, tag=f"xpad{i}",
                           name=f"xpad{i}")
            for s in range(CIH):
                pl = t[:, s, :]
                nc.vector.memset(pl[:, 0:GO + PW + 1], 0.0)
                nc.vector.memset(
                    pl[:, GO + PW:GO + PW + H * PW].rearrange(
                        "p (r c) -> p r c", c=PW)[:, :, 0:1], 0.0)
                nc.vector.memset(
                    pl[:, GO + PW + PW - 1:GO + PW + PW - 1 + H * PW].rearrange(
                        "p (r c) -> p r c", c=PW)[:, :, 0:1], 0.0)
                nc.vector.memset(pl[:, GO + (PW - 1) * PW:PL], 0.0)
            xpad.append(t)
        ident = consts.tile([128, 128], act_dt, tag="ident")
        make_identity(nc, ident)

        wT = consts.tile([128, TAPS, COH, CIH, 128], act_dt, tag="wT")
        scale_alpha = [consts.tile([128, 1], F32, tag=f"sa{h}", name=f"sa{h}")
                       for h in range(COH)]

        # --- weight DMA chunks for one co-half (ci-half x k) ---------------
        HCI = C // CIH * TAPS  # 1152 columns per ci-half
        def dma_chunk(h, ci, k):
            wk = wstage.tile([128, HCI], F32, tag="wsb", name="wk")
            nc.sync.dma_start(
                out=wk,
                in_=w_d.ap()[k, h * 128:(h + 1) * 128,
                             ci * (C // CIH):(ci + 1) * (C // CIH)]
                .rearrange("p c a b -> p (c a b)"))
            return wk

        def dma_half(h, cis=(0, 1)):
            return [dma_chunk(h, ci, k) for ci in cis for k in range(K)]

        # mix (DVE, trailing the DMAs) + sign (ACT), one ci-half at a time
        def mix_ci(ci, wks, wmix, ws):
            for k in range(K):
                wk = wks[k]
                dst = wmix[:, ci * HCI:(ci + 1) * HCI]
                nc.vector.scalar_tensor_tensor(
                    dst, wk, rv[:, k:k + 1], wk if k == 0 else dst,
                    mybir.AluOpType.mult,
                    mybir.AluOpType.bypass if k == 0 else
                    mybir.AluOpType.add)
            nc.scalar.sign(ws[:, ci * HCI:(ci + 1) * HCI],
                           wmix[:, ci * HCI:(ci + 1) * HCI])

        def mix_half(h, wks):
            wmix = wwork.tile([128, C * TAPS], F32, tag="wmix", name="wmix")
            ws = wwork.tile([128, C * TAPS], act_dt, tag=f"wsign{h}", bufs=1,
                            name=f"wsign{h}")
            for ci in range(CIH):
                mix_ci(ci, wks[ci * K:(ci + 1) * K], wmix, ws)
            return ws, wmix

        # |real_w| row-sums + scale*alpha combine, on DVE
        def reduce_half(h, wmix):
            absum = consts.tile([128, 1], F32, tag=f"ab{h}", name=f"ab{h}")
            nc.vector.tensor_reduce(absum, wmix, mybir.AxisListType.X,
                                    mybir.AluOpType.add,
                                    apply_absolute_value=True)
            nc.vector.scalar_tensor_tensor(
                scale_alpha[h], absum, 1.0 / (C * TAPS), alpha_sb[h],
                mybir.AluOpType.mult, mybir.AluOpType.mult)

        # --- transpose one co-half's sign-weights into wT ------------------
        # Two PSUM stages; the PSUM->SBUF copies ride DVE (ACT is busy with
        # sign-x / evacuations around both call sites).
        def transpose_half(h, wsgn, cpsum):
            wsv = wsgn.rearrange("p (ci t) -> p ci t", t=TAPS)
            stages = [("ps1", 3 * 512, 0, 6), ("ps0", 4 * 512, 6, 9)]
            for tag, width, ta, tb in stages:
                tp = cpsum.tile([128, width], F32, tag=tag, bufs=1,
                                name=f"t{tag}")
                for i, (tap, ci) in enumerate(
                        [(t, c) for t in range(ta, tb) for c in range(CIH)]):
                    nc.tensor.matmul(
                        tp[:, i * 128:(i + 1) * 128],
                        wsv[:, ci * 128:(ci + 1) * 128, tap], ident,
                        start=True, stop=True)
                nc.vector.tensor_copy(
                    wT[:, ta:tb, h, :, :],
                    tp[:, 0:(tb - ta) * CIH * 128].rearrange(
                        "p (t ci co) -> p t ci co", t=tb - ta, co=128))

        # --- load one image in two row bands (one DMA per band) ------------
        def load_band(b, r0, r1, bandtag):
            xs = xin.tile([128, CIH, (r1 - r0) * W], F32,
                          tag=f"x{bandtag}", name="xsb")
            nc.sync.dma_start(
                out=xs, in_=x_d.ap()[b, :, r0:r1]
                .rearrange("(s p) a b -> p s (a b)", s=CIH))
            return (r0, r1, xs)

        def load(b):
            return [load_band(b, 0, XA_ROWS, "a"),
                    load_band(b, XA_ROWS, H, "b")]

        def sign_band(b, band):
            r0, r1, src = band
            xp = xpad[b]
            for s in range(CIH):
                dst = xp[:, s, GO:GO + PLANE].rearrange(
                    "p (y x) -> p y x", x=PW)[:, 1 + r0:1 + r1, 1:57]
                nc.scalar.sign(dst, src[:, s, :]
                               .rearrange("p (y x) -> p y x", x=W))

        # --- conv for one (image, co-half) ---------------------------------
        # psum tiles: ps1 (3 chunks) first — it only needs x band A.
        # between the two tiles, `between()` emits next-image work (sign of
        # the following image's band) so ACT ops queue in gate order.
        # defer=True skips the output DMA (issued later, off the congested
        # front window); the caller flushes via out_dma().
        def out_dma(b, h, osb, nch, c0, eng):
            eng.dma_start(
                out=o_d.ap()[b, h * 128:(h + 1) * 128,
                             c0 * RPC:(c0 + nch) * RPC, :].rearrange(
                    "p a b -> p (a b)"),
                in_=osb[:, c0 * RPC * W:(c0 + nch) * RPC * W])

        def conv(b, h, cpsum, between=(None, None), defer=False):
            xp = xpad[b]
            osb = outp.tile([128, H * W], BF16, tag="osb", name="osb")
            for ti, (tag, nch, c0) in enumerate((("ps1", 3, 0),
                                                 ("ps0", 4, 3))):
                ps = cpsum.tile([128, nch * 512], F32, tag=tag, bufs=1,
                                name=tag)
                for itap in range(TAPS):
                    dy, dx = itap // KS - 1, itap % KS - 1
                    lhsT = wT[:, itap, h, :, :]
                    for j in range(nch):
                        c = c0 + j
                        off = GO + (1 + RPC * c + dy) * PW + dx
                        o = ps[:, j * 512:j * 512 + CHUNK]
                        nc.tensor.matmul(
                            o, lhsT, xp[:, :, off:off + CHUNK],
                            start=(itap == 0), stop=(itap == TAPS - 1),
                            perf_mode=mybir.MatmulPerfMode.DoubleRow)
                src = ps.rearrange("p (c e) -> p c e", e=512)[
                    :, 0:nch, 0:CHUNK].rearrange(
                    "p c (r x) -> p c r x", x=PW)[:, :, :, 1:57]
                dst = osb.rearrange("p (y x) -> p y x", x=W)[
                    :, c0 * RPC:(c0 + nch) * RPC, :].rearrange(
                    "p (c r) x -> p c r x", r=RPC)
                nc.scalar.activation(dst, src,
                                     mybir.ActivationFunctionType.Copy,
                                     bias=0.0, scale=scale_alpha[h])
                if not defer:
                    out_dma(b, h, osb, nch, c0, nc.scalar)
                if between[ti] is not None:
                    between[ti]()
            return osb

        # --- schedule ------------------------------------------------------
        # sync-ring DMA order: w-h0, x0, x1, w-h1[2 chunks], x2, x3,
        # w-h1[6 chunks]: the h1 weights are needed only at ~60% of the
        # kernel, so they yield to the x images with tight deadlines.
        with tc.tile_pool(name="cpsum", bufs=1, space="PSUM") as cpsum:
            wks0 = dma_half(0)
            # mix/sign of the weights is emitted BEFORE the x0 signs: the
            # weights are first on the sync ring, so their ACT ops gate
            # earlier and must queue earlier (ACT is strict FIFO).
            ws0, wm0 = mix_half(0, wks0)
            xt = {0: load(0)}
            sign_band(0, xt[0][0])
            sign_band(0, xt[0][1])
            # HAM warmup: fp32 matmuls gated on a mid-stream weight chunk
            # bridge the PE activity window until the transposes arrive, so
            # the clock gate is open (2.4GHz) when the convs start.
            for i in range(8):
                wtp = cpsum.tile([128, 512], F32, tag="tps", bufs=1,
                                 name="warm")
                nc.tensor.matmul(wtp[:, 0:464], wks0[3][:, 0:128],
                                 wks0[3][:, 0:464], start=True, stop=True)
            transpose_half(0, ws0, cpsum)
            reduce_half(0, wm0)

            xt[1] = load(1)

            # h0 pass over all images; prefetch pattern keeps the ACT queue
            # in gate order: evacA(b), sign(b+1,A), evacB(b), sign(b+1,B).
            # The h1 weight chunks + their mixes interleave at points where
            # their expected completion matches the surrounding ACT gates.
            wm1 = wwork.tile([128, C * TAPS], F32, tag="wmix", name="wmix1")
            ws1 = wwork.tile([128, C * TAPS], act_dt, tag="wsign1", bufs=1,
                             name="wsign1")
            osbs = []
            for b in range(BL):
                if b == 1:
                    wks1a = dma_half(1, cis=(0,))   # first h1 ci-half
                    xt[2] = load(2)
                    mix_ci(0, wks1a, wm1, ws1)
                if b == 2:
                    xt[3] = load(3)
                    wks1b = dma_half(1, cis=(1,))   # second h1 ci-half
                if b == 3:
                    mix_ci(1, wks1b, wm1, ws1)
                    reduce_half(1, wm1)
                nb = b + 1
                between = (None, None)
                if nb < BL:
                    between = (lambda i=nb: sign_band(i, xt[i][0]),
                               lambda i=nb: sign_band(i, xt[i][1]))
                osbs.append(conv(b, 0, cpsum, between=between, defer=True))
            transpose_half(1, ws1, cpsum)
            # flush the deferred h0 output DMAs on the (idle) gpsimd ring —
            # off the DMA-congested front window
            for b in range(BL):
                for nch, c0 in ((3, 0), (4, 3)):
                    out_dma(b, 0, osbs[b], nch, c0, nc.gpsimd)
            for b in range(BL):
                conv(b, 1, cpsum)
    nc.compile()
    return nc


def _get_nc():
    if "nc" not in _cache:
        _cache["nc"] = _build()
    return _cache["nc"]


def run(inputs, trace=False):
    nc = _get_nc()
    x = np.ascontiguousarray(inputs["x"], dtype=np.float32)
    in_maps = [
        {
            "x": x[c * BL:(c + 1) * BL],
            "weights": np.ascontiguousarray(inputs["weights"], np.float32),
            "RV": np.ascontiguousarray(inputs["RV"], np.float32),
            "alpha": np.ascontiguousarray(inputs["alpha"], np.float32),
        }
        for c in range(NCORES)
    ]
    res = run_bass_kernel_spmd(nc, in_maps, core_ids=list(range(NCORES)),
                               trace=trace)
    out = np.concatenate([np.asarray(r["out"]).astype(np.float32)
                          for r in res.results], axis=0)
    return out, res


def kernel(**inputs) -> np.ndarray:
    out, _ = run(inputs, trace=False)
    return out


# revision 12
# speedup vs baseline: 1.0724x; 1.0724x over previous
"""Binarized conv2d kernel for Trainium2, SPMD over 8 NeuronCores.

Math (forward-value equivalent of the reference):
    real_w  = sum_k RV[k] * weights[k]          # [256,256,3,3], exact fp32 on DVE
    scale   = mean(|real_w|, axis=(1,2,3))      # per out-channel
    out     = conv2d(sign(x), sign(real_w), pad=1) * (scale * alpha)

sign(x) and sign(real_w) are {-1,0,+1} which are exact in fp8e4, so the conv
is computed with fp8 DoubleRow matmuls (exact integer accumulation in fp32
PSUM) and the per-channel scale*alpha is applied on PSUM evacuation.

Sharding: data-parallel over batch, 4 images per core; weights/RV/alpha
replicated. No collectives (an 8-core AllGather measures ~80us on this
fabric — host-proxied — so TP weight prep is a net loss).

Key layout: the padded sign(x) plane stores the two ci-halves ROW-
INTERLEAVED — element (row, s, col) at row*116 + s*58 + col — so a conv
matmul's rhs is a tight 4D AP [128, s=2, rows=8, 56]: the DoubleRow matmul
contracts both ci-halves, reads only the 56 real columns (no pad-column
waste), and its dependency bounding box covers just the 8-row window, which
makes the banded x loads actually overlap with the convs.

Schedule: the front is DMA-bound (~22MB of weights+x reads at the
~350-420GB/s per-core ceiling), so the conv passes run h0 for ALL four
images first, then h1: the h1 weight half is interleaved into the x stream
and only needed at ~60% of the kernel.  Outputs are written bf16 (halves
write traffic; well inside the 2e-2 tolerance), upcast to f32 on the host,
and the h0-pass output DMAs are deferred past the congested front window.
"""

import numpy as np
from contextlib import ExitStack

import concourse.bass as bass
import concourse.bacc as bacc
import concourse.tile as tile
from concourse import mybir
from concourse.bass_utils import run_bass_kernel_spmd
from concourse.masks import make_identity

# Problem shapes (hardcoded per contract)
B, C, H, W = 32, 256, 56, 56
K, KS = 4, 3
NCORES = 8
BL = B // NCORES            # images per core

PW = W + 2                  # padded width 58
PR = H + 2                  # padded rows 58
CIH = C // 128              # 2 ci halves
SROW = CIH * PW             # 116: interleaved row stride
COH = C // 128              # 2 co halves
TAPS = KS * KS              # 9
RPC = 8                     # rows per chunk
CHUNK = RPC * W             # 448 psum elems per matmul chunk
NCHUNK = H // RPC           # 7 chunks: first psum tile gets 3, second 4

# rows of x covered by the first conv tile (chunks 0-2 + halo)
XA_ROWS = 3 * RPC + 1       # 25

F32 = mybir.dt.float32
FP8 = mybir.dt.float8e4
BF16 = mybir.dt.bfloat16

_cache = {}


def _build():
    act_dt = FP8
    nc = bacc.Bacc("TRN2", target_bir_lowering=False, debug=False,
                   num_devices=NCORES)
    x_d = nc.dram_tensor("x", [BL, C, H, W], F32, kind="ExternalInput")
    w_d = nc.dram_tensor("weights", [K, C, C, KS, KS], F32, kind="ExternalInput")
    rv_d = nc.dram_tensor("RV", [K + 1], F32, kind="ExternalInput")
    al_d = nc.dram_tensor("alpha", [C, 1, 1], F32, kind="ExternalInput")
    o_d = nc.dram_tensor("out", [BL, C, H, W], BF16, kind="ExternalOutput")

    with tile.TileContext(nc) as tc, ExitStack() as ctx:
        consts = ctx.enter_context(tc.tile_pool(name="consts", bufs=1))
        wstage = ctx.enter_context(tc.tile_pool(name="wstage", bufs=8))
        wwork = ctx.enter_context(tc.tile_pool(name="wwork", bufs=1))
        xin = ctx.enter_context(tc.tile_pool(name="xin", bufs=2))
        xpads = ctx.enter_context(tc.tile_pool(name="xpads", bufs=1))
        # 6 bufs: the 4 h0-pass outputs are held in SBUF until their
        # deferred DMAs fire, plus 2 rotating for the h1 pass
        outp = ctx.enter_context(tc.tile_pool(name="outp", bufs=6))

        # --- tiny constant loads on the ACT HWDGE ring ---------------------
        rv = consts.tile([128, K], F32, tag="rv")
        rv_src = bass.AP(tensor=rv_d.ap().tensor, offset=0,
                         ap=[[0, 128], [1, K]])
        nc.scalar.dma_start(out=rv, in_=rv_src)
        alpha_sb = []
        for h in range(COH):
            t = consts.tile([128, 1], F32, tag=f"alpha{h}")
            nc.scalar.dma_start(out=t,
                                in_=al_d.ap()[h * 128:(h + 1) * 128, 0, :])
            alpha_sb.append(t)

        # --- padded x planes (one per image, row-interleaved ci-halves):
        # zero only the pad borders (DVE), covering both halves per op
        xpad = []
        for i in range(BL):
            t = xpads.tile([128, PR, CIH, PW], act_dt, tag=f"xpad{i}",
                           name=f"xpad{i}")
            nc.vector.memset(t[:, 0, :, :], 0.0)       # top pad row
            nc.vector.memset(t[:, PR - 1, :, :], 0.0)  # bottom pad row
            nc.vector.memset(t[:, 1:PR - 1, :, 0], 0.0)       # left pad col
            nc.vector.memset(t[:, 1:PR - 1, :, PW - 1], 0.0)  # right pad col
            xpad.append(t)
        ident = consts.tile([128, 128], act_dt, tag="ident")
        make_identity(nc, ident)

        wT = consts.tile([128, TAPS, COH, CIH, 128], act_dt, tag="wT")
        scale_alpha = [consts.tile([128, 1], F32, tag=f"sa{h}", name=f"sa{h}")
                       for h in range(COH)]

        # --- weight DMA chunks for one co-half (ci-half x k) ---------------
        HCI = C // CIH * TAPS  # 1152 columns per ci-half
        def dma_half(h, cis=(0, 1)):
            wks = []
            for ci in cis:
                for k in range(K):
                    wk = wstage.tile([128, HCI], F32, tag="wsb", name="wk")
                    wks.append(wk)
                    nc.sync.dma_start(
                        out=wk,
                        in_=w_d.ap()[k, h * 128:(h + 1) * 128,
                                     ci * (C // CIH):(ci + 1) * (C // CIH)]
                        .rearrange("p c a b -> p (c a b)"))
            return wks

        # mix (DVE, trailing the DMAs) + sign (ACT), one ci-half at a time
        def mix_ci(ci, wks, wmix, ws):
            for k in range(K):
                wk = wks[k]
                dst = wmix[:, ci * HCI:(ci + 1) * HCI]
                nc.vector.scalar_tensor_tensor(
                    dst, wk, rv[:, k:k + 1], wk if k == 0 else dst,
                    mybir.AluOpType.mult,
                    mybir.AluOpType.bypass if k == 0 else
                    mybir.AluOpType.add)
            nc.scalar.sign(ws[:, ci * HCI:(ci + 1) * HCI],
                           wmix[:, ci * HCI:(ci + 1) * HCI])

        def mix_half(h, wks):
            wmix = wwork.tile([128, C * TAPS], F32, tag=f"wmix{h}",
                              name=f"wmix{h}")
            ws = wwork.tile([128, C * TAPS], act_dt, tag=f"wsign{h}", bufs=1,
                            name=f"wsign{h}")
            for ci in range(CIH):
                mix_ci(ci, wks[ci * K:(ci + 1) * K], wmix, ws)
            return ws, wmix

        # |real_w| row-sums + scale*alpha combine, on DVE
        def reduce_half(h, wmix):
            absum = consts.tile([128, 1], F32, tag=f"ab{h}", name=f"ab{h}")
            nc.vector.tensor_reduce(absum, wmix, mybir.AxisListType.X,
                                    mybir.AluOpType.add,
                                    apply_absolute_value=True)
            nc.vector.scalar_tensor_tensor(
                scale_alpha[h], absum, 1.0 / (C * TAPS), alpha_sb[h],
                mybir.AluOpType.mult, mybir.AluOpType.mult)

        # --- transpose one co-half's sign-weights into wT ------------------
        # Two PSUM stages; the PSUM->SBUF copies ride DVE (ACT is busy with
        # sign-x / evacuations around both call sites).
        def transpose_half(h, wsgn, cpsum):
            wsv = wsgn.rearrange("p (ci t) -> p ci t", t=TAPS)
            stages = [("ps1", 3 * 512, 0, 6), ("ps0", 4 * 512, 6, 9)]
            for tag, width, ta, tb in stages:
                tp = cpsum.tile([128, width], F32, tag=tag, bufs=1,
                                name=f"t{tag}")
                for i, (tap, ci) in enumerate(
                        [(t, c) for t in range(ta, tb) for c in range(CIH)]):
                    nc.tensor.matmul(
                        tp[:, i * 128:(i + 1) * 128],
                        wsv[:, ci * 128:(ci + 1) * 128, tap], ident,
                        start=True, stop=True)
                nc.vector.tensor_copy(
                    wT[:, ta:tb, h, :, :],
                    tp[:, 0:(tb - ta) * CIH * 128].rearrange(
                        "p (t ci co) -> p t ci co", t=tb - ta, co=128))

        # --- load one image in two row bands (one DMA per band) ------------
        def load_band(b, r0, r1, bandtag):
            xs = xin.tile([128, CIH, (r1 - r0) * W], F32,
                          tag=f"x{bandtag}", name="xsb")
            nc.sync.dma_start(
                out=xs, in_=x_d.ap()[b, :, r0:r1]
                .rearrange("(s p) a b -> p s (a b)", s=CIH))
            return (r0, r1, xs)

        def load(b):
            return [load_band(b, 0, XA_ROWS, "a"),
                    load_band(b, XA_ROWS, H, "b")]

        def sign_band(b, band):
            r0, r1, src = band
            xp = xpad[b]
            for s in range(CIH):
                dst = xp[:, 1 + r0:1 + r1, s, 1:57]
                nc.scalar.sign(dst, src[:, s, :]
                               .rearrange("p (y x) -> p y x", x=W))

        # --- conv for one (image, co-half) ---------------------------------
        # psum tiles: ps1 (3 chunks) first — it only needs x band A.
        # between the two tiles, `between()` emits next-image work (sign of
        # the following image's band) so ACT ops queue in gate order.
        # defer=True skips the output DMA (issued later, off the congested
        # front window); the caller flushes via out_dma().
        def out_dma(b, h, osb, nch, c0, eng):
            eng.dma_start(
                out=o_d.ap()[b, h * 128:(h + 1) * 128,
                             c0 * RPC:(c0 + nch) * RPC, :].rearrange(
                    "p a b -> p (a b)"),
                in_=osb[:, c0 * CHUNK:(c0 + nch) * CHUNK])

        def conv(b, h, cpsum, between=(None, None), defer=False):
            xf = xpad[b].rearrange("p a s c -> p (a s c)")
            pdim = xf.ap[0]
            osb = outp.tile([128, H * W], BF16, tag="osb", name="osb")
            for ti, (tag, nch, c0) in enumerate((("ps1", 3, 0),
                                                 ("ps0", 4, 3))):
                ps = cpsum.tile([128, nch * 512], F32, tag=tag, bufs=1,
                                name=tag)
                for itap in range(TAPS):
                    dy, dx = itap // KS - 1, itap % KS - 1
                    lhsT = wT[:, itap, h, :, :]
                    for j in range(nch):
                        c = c0 + j
                        base = (1 + RPC * c + dy) * SROW + 1 + dx
                        rhs = bass.AP(
                            tensor=xf.tensor, offset=xf.offset + base,
                            ap=[pdim, [PW, CIH], [SROW, RPC], [1, W]])
                        nc.tensor.matmul(
                            ps[:, j * 512:j * 512 + CHUNK], lhsT, rhs,
                            start=(itap == 0), stop=(itap == TAPS - 1),
                            perf_mode=mybir.MatmulPerfMode.DoubleRow)
                src = ps.rearrange("p (c e) -> p c e", e=512)[:, :, 0:CHUNK]
                dst = osb[:, c0 * CHUNK:(c0 + nch) * CHUNK].rearrange(
                    "p (c e) -> p c e", e=CHUNK)
                nc.scalar.activation(dst, src,
                                     mybir.ActivationFunctionType.Copy,
                                     bias=0.0, scale=scale_alpha[h])
                if not defer:
                    out_dma(b, h, osb, nch, c0, nc.scalar)
                if between[ti] is not None:
                    between[ti]()
            return osb

        # --- schedule ------------------------------------------------------
        # sync-ring DMA order: w-h0, x0, x1, w-h1[ci0], x2, x3, w-h1[ci1]:
        # the h1 weights are only needed at ~60% of the kernel, so they
        # yield to the x images with tight deadlines.
        with tc.tile_pool(name="cpsum", bufs=1, space="PSUM") as cpsum:
            wks0 = dma_half(0)
            # mix/sign of the weights is emitted BEFORE the x0 signs: the
            # weights are first on the sync ring, so their ACT ops gate
            # earlier and must queue earlier (ACT is strict FIFO).
            ws0, wm0 = mix_half(0, wks0)
            xt = {0: load(0)}
            sign_band(0, xt[0][0])
            sign_band(0, xt[0][1])
            # HAM warmup: fp32 matmuls gated on a mid-stream weight chunk
            # bridge the PE activity window until the transposes arrive, so
            # the clock gate is open (2.4GHz) when the convs start.
            for i in range(8):
                wtp = cpsum.tile([128, 512], F32, tag="tps", bufs=1,
                                 name="warm")
                nc.tensor.matmul(wtp[:, 0:464], wks0[3][:, 0:128],
                                 wks0[3][:, 0:464], start=True, stop=True)
            transpose_half(0, ws0, cpsum)
            reduce_half(0, wm0)

            xt[1] = load(1)

            # h0 pass over all images; prefetch pattern keeps the ACT queue
            # in gate order: evacA(b), sign(b+1,A), evacB(b), sign(b+1,B).
            wm1 = wwork.tile([128, C * TAPS], F32, tag="wmix1", name="wmix1")
            ws1 = wwork.tile([128, C * TAPS], act_dt, tag="wsign1", bufs=1,
                             name="wsign1")
            osbs = []
            for b in range(BL):
                if b == 1:
                    wks1a = dma_half(1, cis=(0,))   # first h1 ci-half
                    xt[2] = load(2)
                    mix_ci(0, wks1a, wm1, ws1)
                if b == 2:
                    xt[3] = load(3)
                    wks1b = dma_half(1, cis=(1,))   # second h1 ci-half
                if b == 3:
                    mix_ci(1, wks1b, wm1, ws1)
                    reduce_half(1, wm1)
                nb = b + 1
                between = (None, None)
                if nb < BL:
                    between = (lambda i=nb: sign_band(i, xt[i][0]),
                               lambda i=nb: sign_band(i, xt[i][1]))
                osbs.append(conv(b, 0, cpsum, between=between, defer=True))
            transpose_half(1, ws1, cpsum)
            # flush the deferred h0 output DMAs on the (idle) gpsimd ring —
            # off the DMA-congested front window
            for b in range(BL):
                for nch, c0 in ((3, 0), (4, 3)):
                    out_dma(b, 0, osbs[b], nch, c0, nc.gpsimd)
            for b in range(BL):
                conv(b, 1, cpsum)
    nc.compile()
    return nc


def _get_nc():
    if "nc" not in _cache:
        _cache["nc"] = _build()
    return _cache["nc"]


def run(inputs, trace=False):
    nc = _get_nc()
    x = np.ascontiguousarray(inputs["x"], dtype=np.float32)
    in_maps = [
        {
            "x": x[c * BL:(c + 1) * BL],
            "weights": np.ascontiguousarray(inputs["weights"], np.float32),
            "RV": np.ascontiguousarray(inputs["RV"], np.float32),
            "alpha": np.ascontiguousarray(inputs["alpha"], np.float32),
        }
        for c in range(NCORES)
    ]
    res = run_bass_kernel_spmd(nc, in_maps, core_ids=list(range(NCORES)),
                               trace=trace)
    out = np.concatenate([np.asarray(r["out"]).astype(np.float32)
                          for r in res.results], axis=0)
    return out, res


def kernel(**inputs) -> np.ndarray:
    out, _ = run(inputs, trace=False)
    return out
